# revision 3
# baseline (speedup 1.0000x reference)
"""Trainium2 Bass kernel for BlankEmbedding (embedding lookup + blank shift-accumulate).

Reference semantics:
    out = emb[x]                               # [B, S, D]
    preblank[s] = (x[s+1]==BLANK) & (x[s]!=BLANK)   (per row; preblank[S-1]=0)
    out[s] += sum_{k=1..3} preblank[s-k] * emb[x[s-k]]   (zero-pad at row start)

Strategy: data-parallel over the 16384 flattened tokens, 2048 per core.
Each core holds the full table in DRAM and gathers its 2048 rows with
per-partition-index indirect DMAs (16 tiles of [128, DIM], token t =
128*i + p). The preblank mask is computed on-device from an int32 token
stream (with a 3-token halo so runs crossing core boundaries are
handled; the halo is blank-filled at row starts, which forces the mask
to 0 there, matching the reference zero-padding). The shift-accumulate
becomes 3 masked gather-adds per tile: indices of non-preblank
positions are pushed out of bounds and skipped via bounds_check +
oob_is_err=False, while valid rows are added in the DMA datapath
(compute_op=add). Finally each tile is stored contiguously.
"""

import numpy as np

VOCAB = 50257
DIM = 1024
BLANK = 100
B, S = 4, 4096
N_CORES = 8
TOK = B * S                  # 16384 flattened tokens
TPC = TOK // N_CORES         # 2048 tokens per core
P = 128                      # SBUF partitions
NT = TPC // P                # 16 tiles per core
HALO = 3                     # max shift distance
EXT = TPC + HALO             # 2051 extended token stream length
MLEN = EXT - 1               # 2050 masked-index stream length
BIG = 1 << 20                # pushes masked-out indices beyond bounds_check

_CACHE = {}


def _build_nc():
    from concourse import bacc, mybir, tile
    import concourse.bass as bass

    nc = bacc.Bacc(
        "TRN2", target_bir_lowering=False, debug=False, num_devices=N_CORES
    )
    i32 = mybir.dt.int32
    f32 = mybir.dt.float32

    idx_ext = nc.dram_tensor("idx_ext", [EXT], i32, kind="ExternalInput")
    emb = nc.dram_tensor("emb", [VOCAB, DIM], f32, kind="ExternalInput")
    out = nc.dram_tensor("out", [TPC, DIM], f32, kind="ExternalOutput")
    m_dram = nc.dram_tensor("m_scratch", [MLEN], i32)

    with tile.TileContext(nc) as tc:
        with tc.tile_pool(name="sbuf", bufs=1) as pool:
            # ---- preblank mask chain on one partition (free-axis shifts) ----
            ix_row = pool.tile([1, EXT], i32)
            nc.sync.dma_start(out=ix_row[:], in_=idx_ext[None, :])

            b_row = pool.tile([1, EXT], i32)  # 1 where token == BLANK
            nc.vector.tensor_scalar(
                out=b_row[:], in0=ix_row[:], scalar1=BLANK, scalar2=None,
                op0=mybir.AluOpType.is_equal,
            )
            nb_row = pool.tile([1, MLEN], i32)  # 1 - b (unshifted view)
            nc.vector.tensor_scalar(
                out=nb_row[:], in0=b_row[:, 0:MLEN], scalar1=-1, scalar2=1,
                op0=mybir.AluOpType.mult, op1=mybir.AluOpType.add,
            )
            w_row = pool.tile([1, MLEN], i32)  # preblank flag per ext position
            nc.vector.tensor_tensor(
                out=w_row[:], in0=b_row[:, 1:EXT], in1=nb_row[:],
                op=mybir.AluOpType.mult,
            )
            # masked index stream: idx where preblank, idx+BIG (OOB) elsewhere
            m_row = pool.tile([1, MLEN], i32)
            nc.vector.tensor_scalar(
                out=m_row[:], in0=w_row[:], scalar1=-BIG, scalar2=BIG,
                op0=mybir.AluOpType.mult, op1=mybir.AluOpType.add,
            )
            nc.vector.tensor_tensor(
                out=m_row[:], in0=m_row[:], in1=ix_row[:, 0:MLEN],
                op=mybir.AluOpType.add,
            )
            # roundtrip through DRAM to relayout free-axis -> partition-major
            nc.sync.dma_start(out=m_dram[None, :], in_=m_row[:])

            # ---- index tiles ----
            ix_sb = [pool.tile([P, 1], i32, tag=f"ix{i}", name=f"ix{i}") for i in range(NT)]
            for i in range(NT):
                nc.sync.dma_start(
                    out=ix_sb[i][:],
                    in_=idx_ext[HALO + P * i : HALO + P * (i + 1), None],
                )
            mk_sb = {}
            for k in (1, 2, 3):
                for i in range(NT):
                    t = pool.tile([P, 1], i32, tag=f"mk{k}_{i}", name=f"mk{k}_{i}")
                    base = HALO - k + P * i
                    nc.sync.dma_start(
                        out=t[:], in_=m_dram[base : base + P, None]
                    )
                    mk_sb[(k, i)] = t

            # ---- main gathers: token t = 128*i + p -> g[i][p, :] ----
            g = [pool.tile([P, DIM], f32, tag=f"g{i}", name=f"g{i}") for i in range(NT)]
            for i in range(NT):
                nc.gpsimd.indirect_dma_start(
                    out=g[i][:], out_offset=None, in_=emb[:],
                    in_offset=bass.IndirectOffsetOnAxis(
                        ap=ix_sb[i][:, :1], axis=0
                    ),
                )

            # ---- masked gather-adds for the 3 shift distances ----
            for k in (1, 2, 3):
                for i in range(NT):
                    nc.gpsimd.indirect_dma_start(
                        out=g[i][:], out_offset=None, in_=emb[:],
                        in_offset=bass.IndirectOffsetOnAxis(
                            ap=mk_sb[(k, i)][:, :1], axis=0
                        ),
                        bounds_check=VOCAB - 1,
                        oob_is_err=False,
                        compute_op=mybir.AluOpType.add,
                    )

            # ---- stores ----
            for i in range(NT):
                nc.sync.dma_start(
                    out=out[P * i : P * (i + 1), :], in_=g[i][:]
                )

    nc.compile()
    return nc


def get_nc():
    if "nc" not in _CACHE:
        _CACHE["nc"] = _build_nc()
    return _CACHE["nc"]


def shard_inputs(x, emb_table):
    """Build per-core in_maps from full inputs."""
    flat = np.ascontiguousarray(np.asarray(x).astype(np.int32).reshape(-1))
    emb_f32 = np.ascontiguousarray(np.asarray(emb_table, dtype=np.float32))
    in_maps = []
    for c in range(N_CORES):
        start = c * TPC
        ext = np.empty(EXT, dtype=np.int32)
        if start % S == 0:
            # row start: blank-filled halo makes the preblank mask 0 there,
            # matching the reference's zero-padded shifts at row boundaries
            ext[:HALO] = BLANK
        else:
            ext[:HALO] = flat[start - HALO : start]
        ext[HALO:] = flat[start : start + TPC]
        in_maps.append({"idx_ext": ext, "emb": emb_f32})
    return in_maps


def assemble_output(results):
    parts = [results[c]["out"] for c in range(N_CORES)]
    return np.concatenate(parts, axis=0).reshape(B, S, DIM)


def kernel(x, emb_table):
    from concourse.bass_utils import run_bass_kernel_spmd

    nc = get_nc()
    in_maps = shard_inputs(x, emb_table)
    res = run_bass_kernel_spmd(nc, in_maps, core_ids=list(range(N_CORES)))
    return assemble_output(res.results)


# revision 4
# speedup vs baseline: 1.3266x; 1.3266x over previous
"""Trainium2 Bass kernel for BlankEmbedding (embedding lookup + blank shift-accumulate).

Reference semantics:
    out = emb[x]                               # [B, S, D]
    preblank[s] = (x[s+1]==BLANK) & (x[s]!=BLANK)   (per row; preblank[S-1]=0)
    out[s] += sum_{k=1..3} preblank[s-k] * emb[x[s-k]]   (zero-pad at row start)

Strategy: data-parallel over the 16384 flattened tokens, 2048 per core.
Each core holds the full table in DRAM and gathers its 2048 rows with
per-partition-index indirect DMAs (16 tiles of [128, DIM], token
t = 128*i + p; SWDGE indirect DMA is limited to one index per
partition per instruction on HW, and each indirect instruction costs
~1.8us of GPSIMD descriptor-generation time, so the instruction count
is kept minimal: 16 + 1 halo gather).

The shift-accumulate is done on-chip with the tensor engine: for each
tile, c_i = (A*w_i).T @ g_i + (E*w_{i-1}).T @ g_{i-1}, where A/E are
constant shifted-identity band matrices (A[q,p]=1 iff 1<=p-q<=3 within
the tile, E[q,p]=1 iff 1<=p+128-q<=3 for the tile boundary) and w is
the per-position preblank mask computed on-device from the int32 token
stream. A 3-token halo (tile "-1") covers runs crossing core
boundaries; the halo is blank-filled at row starts, which forces the
mask to 0 there, matching the reference zero-padding. out_i = g_i + c_i
is fused into one DVE add from PSUM, stored with plain HWDGE DMAs.
"""

import numpy as np

VOCAB = 50257
DIM = 1024
BLANK = 100
B, S = 4, 4096
N_CORES = 8
TOK = B * S                  # 16384 flattened tokens
TPC = TOK // N_CORES         # 2048 tokens per core
P = 128                      # SBUF partitions
NT = TPC // P                # 16 tiles per core
HALO = 3                     # max shift distance
EXT = TPC + HALO + 1         # 2052: 3 halo + 2048 tokens + 1 pad
NMM = DIM // 512             # matmul free-dim chunks per tile

_CACHE = {}


def _shift_consts():
    """A[q,p]=1 iff 1<=p-q<=3 (in-tile shifts); E[q,p]=1 iff 1<=p+128-q<=3
    (shifts crossing the tile boundary, sources in the previous tile)."""
    q = np.arange(P)[:, None]
    p = np.arange(P)[None, :]
    a_mat = ((p - q >= 1) & (p - q <= HALO)).astype(np.float32)
    e_mat = ((p + P - q >= 1) & (p + P - q <= HALO)).astype(np.float32)
    return a_mat, e_mat


def _build_nc():
    from concourse import bacc, mybir, tile
    import concourse.bass as bass

    nc = bacc.Bacc(
        "TRN2", target_bir_lowering=False, debug=False, num_devices=N_CORES
    )
    i32 = mybir.dt.int32
    f32 = mybir.dt.float32

    idx_ext = nc.dram_tensor("idx_ext", [EXT], i32, kind="ExternalInput")
    emb = nc.dram_tensor("emb", [VOCAB, DIM], f32, kind="ExternalInput")
    a_dram = nc.dram_tensor("a_mat", [P, P], f32, kind="ExternalInput")
    e_dram = nc.dram_tensor("e_mat", [P, P], f32, kind="ExternalInput")
    out = nc.dram_tensor("out", [TPC, DIM], f32, kind="ExternalOutput")

    with tile.TileContext(nc) as tc:
        with (
            tc.tile_pool(name="sbuf", bufs=1) as pool,
            tc.tile_pool(name="psum", bufs=3, space="PSUM") as psum_pool,
        ):
            a_sb = pool.tile([P, P], f32)
            e_sb = pool.tile([P, P], f32)
            nc.scalar.dma_start(out=a_sb[:], in_=a_dram[:])
            nc.scalar.dma_start(out=e_sb[:], in_=e_dram[:])

            # ---- halo tile (tile "-1"): partitions 125..127 = the 3 halo
            # positions; lower partitions are masked out by w=0 ----
            ix_h = pool.tile([P, 1], i32)
            ixn_h = pool.tile([P, 1], i32)
            nc.vector.memset(ix_h[:], 0)
            nc.vector.memset(ixn_h[:], 0)
            nc.scalar.dma_start(out=ix_h[P - HALO :, :], in_=idx_ext[0:HALO, None])
            nc.scalar.dma_start(
                out=ixn_h[P - HALO :, :], in_=idx_ext[1 : HALO + 1, None]
            )

            ix = [ix_h]    # per-tile gather indices (tile -1 at [0])
            ixn = [ixn_h]  # next-token indices
            for i in range(NT):
                t = pool.tile([P, 1], i32, name=f"ix{i}", tag=f"ix{i}")
                nc.scalar.dma_start(
                    out=t[:], in_=idx_ext[HALO + P * i : HALO + P * (i + 1), None]
                )
                ix.append(t)
                t = pool.tile([P, 1], i32, name=f"ixn{i}", tag=f"ixn{i}")
                nc.scalar.dma_start(
                    out=t[:],
                    in_=idx_ext[HALO + 1 + P * i : HALO + 1 + P * (i + 1), None],
                )
                ixn.append(t)

            # ---- preblank masks w[p] = isblank(next) & ~isblank(cur) ----
            w = []
            for j in range(NT + 1):
                b = pool.tile([P, 1], i32, name=f"b{j}", tag=f"b{j}")
                bn = pool.tile([P, 1], i32, name=f"bn{j}", tag=f"bn{j}")
                nc.vector.tensor_scalar(
                    out=b[:], in0=ix[j][:], scalar1=BLANK, scalar2=None,
                    op0=mybir.AluOpType.is_equal,
                )
                nc.vector.tensor_scalar(
                    out=bn[:], in0=ixn[j][:], scalar1=BLANK, scalar2=None,
                    op0=mybir.AluOpType.is_equal,
                )
                nc.vector.tensor_scalar(  # b := 1 - b
                    out=b[:], in0=b[:], scalar1=-1, scalar2=1,
                    op0=mybir.AluOpType.mult, op1=mybir.AluOpType.add,
                )
                nc.vector.tensor_tensor(  # bn := bn * (1 - b)
                    out=bn[:], in0=bn[:], in1=b[:], op=mybir.AluOpType.mult,
                )
                wf = pool.tile([P, 1], f32, name=f"w{j}", tag=f"w{j}")
                nc.vector.tensor_copy(out=wf[:], in_=bn[:])
                w.append(wf)

            # ---- main gathers (one index per partition per instruction) ----
            g = []
            for j in range(NT + 1):
                t = pool.tile([P, DIM], f32, name=f"g{j}", tag=f"g{j}")
                nc.gpsimd.indirect_dma_start(
                    out=t[:], out_offset=None, in_=emb[:],
                    in_offset=bass.IndirectOffsetOnAxis(ap=ix[j][:, :1], axis=0),
                )
                g.append(t)

            # ---- corrections via shifted-masked-identity matmuls ----
            for i in range(NT):
                aw = pool.tile([P, P], f32, name=f"aw{i}", tag=f"aw{i}")
                ew = pool.tile([P, P], f32, name=f"ew{i}", tag=f"ew{i}")
                nc.vector.tensor_tensor(
                    out=aw[:], in0=a_sb[:],
                    in1=w[i + 1][:].to_broadcast([P, P]),
                    op=mybir.AluOpType.mult,
                )
                nc.vector.tensor_tensor(
                    out=ew[:], in0=e_sb[:],
                    in1=w[i][:].to_broadcast([P, P]),
                    op=mybir.AluOpType.mult,
                )
                c = psum_pool.tile([P, DIM], f32, name=f"c{i}", tag="c")
                for h in range(NMM):
                    sl = slice(512 * h, 512 * (h + 1))
                    nc.tensor.matmul(
                        out=c[:, sl], lhsT=aw[:], rhs=g[i + 1][:, sl],
                        start=True, stop=False,
                    )
                    nc.tensor.matmul(
                        out=c[:, sl], lhsT=ew[:], rhs=g[i][:, sl],
                        start=False, stop=True,
                    )
                nc.vector.tensor_tensor(  # g_i := g_i + c_i
                    out=g[i + 1][:], in0=g[i + 1][:], in1=c[:],
                    op=mybir.AluOpType.add,
                )
                nc.sync.dma_start(
                    out=out[P * i : P * (i + 1), :], in_=g[i + 1][:]
                )

    nc.compile()
    return nc


def get_nc():
    if "nc" not in _CACHE:
        _CACHE["nc"] = _build_nc()
    return _CACHE["nc"]


def shard_inputs(x, emb_table):
    """Build per-core in_maps from full inputs."""
    flat = np.ascontiguousarray(np.asarray(x).astype(np.int32).reshape(-1))
    emb_f32 = np.ascontiguousarray(np.asarray(emb_table, dtype=np.float32))
    a_mat, e_mat = _shift_consts()
    in_maps = []
    for c in range(N_CORES):
        start = c * TPC
        ext = np.zeros(EXT, dtype=np.int32)
        if start % S == 0:
            # row start: blank-filled halo makes the preblank mask 0 there,
            # matching the reference's zero-padded shifts at row boundaries
            ext[:HALO] = BLANK
        else:
            ext[:HALO] = flat[start - HALO : start]
        ext[HALO : HALO + TPC] = flat[start : start + TPC]
        # ext[-1] stays 0: only read to build w at the last position, whose
        # A-matrix row is all-zero (contributions belong to the next core)
        in_maps.append(
            {"idx_ext": ext, "emb": emb_f32, "a_mat": a_mat, "e_mat": e_mat}
        )
    return in_maps


def assemble_output(results):
    parts = [results[c]["out"] for c in range(N_CORES)]
    return np.concatenate(parts, axis=0).reshape(B, S, DIM)


def kernel(x, emb_table):
    from concourse.bass_utils import run_bass_kernel_spmd

    nc = get_nc()
    in_maps = shard_inputs(x, emb_table)
    res = run_bass_kernel_spmd(nc, in_maps, core_ids=list(range(N_CORES)))
    return assemble_output(res.results)
